# revision 2
# baseline (speedup 1.0000x reference)
"""Trainium2 Bass kernel for nn_BlockBucket (3x eres_block + basic_block).

Strategy: the per-pixel dynamic conv (filters from a 72-entry embedding table
indexed by `buckets`) is computed as bucket-sorted matmuls: pixels are grouped
by bucket into 64-pixel tiles (host-side index prep only -- all FLOPs on
device), patches are built with one dma_gather per block from an AllGathered
pixel-major conv1 output, and each tile does 9 accumulating K=64 matmuls
against its bucket's filter.  Everything between the 3x3 convs is pointwise and
stays in the bucket-sorted "slot" domain.  2 AllGathers/block (conv1out image,
o_k slots), none after block 3 (host reassembles from per-core slot outputs).
"""

import sys

sys.path.insert(0, "/opt/trn_rl_repo")

import numpy as np
import ml_dtypes

BF16 = ml_dtypes.bfloat16

# problem constants
C = 64
H = W = 64
NPIX = H * W            # 4096
NTYPES = 72
KK = 9                  # 3x3
EMB_DIM = C * (C * KK + 1)
GROUP = 4
NCORES = 8

# layout constants
TS = 64                 # slot tile size
S_MAX = 12              # tiles per core (supports up to 96 tiles globally)
S = S_MAX * TS          # 768 slots per core
S_PAD = S + 64          # +64 guaranteed-zero rows in the AG2 contribution
ZERO_SLOT = S           # rank-0 row index of a zero row (global row = S)
ROWS_PER_CORE = H // NCORES     # 8 image rows per strip
PIX_PER_CORE = ROWS_PER_CORE * W  # 512
PW = 66                 # padded row width for strip layout
STRIP_R = 12            # strip tile rows: 8 interior + 2 halo + 2 margin
IG_N = 896              # image-gather num_idxs (>= STRIP_R*PW=792, %128==0)
PG_N = KK * S           # patch-gather num_idxs = 6912 (%128==0)


def _reflect(v, n=64):
    if v < 0:
        return -v
    if v >= n:
        return 2 * n - 2 - v
    return v


def _wrap_idx(idx, n):
    """int16 index array -> [128, n//16] layout (j -> [j%16, j//16])."""
    assert len(idx) == n and n % 16 == 0
    blk = np.asarray(idx, np.int16).reshape(n // 16, 16).T
    return np.tile(blk, (8, 1))


def _host_prep(x, buckets, params):
    """Build per-core input maps + assembly info. All numpy."""
    x = np.asarray(x, np.float32).reshape(C, NPIX)
    bk = np.asarray(buckets, np.int64).reshape(NPIX)

    # ---- slot assignment (shared by all 3 blocks) ----
    tiles = []  # list of (bucket, [pixels padded with -1 to TS])
    for t in range(NTYPES):
        pix = np.nonzero(bk == t)[0]
        for off in range(0, max(len(pix), 1), TS):
            chunk = pix[off : off + TS]
            if len(chunk) == 0:
                continue
            pad = np.full(TS, -1, np.int64)
            pad[: len(chunk)] = chunk
            tiles.append((t, pad))
    n_tiles = len(tiles)
    assert n_tiles <= NCORES * S_MAX, f"too many tiles: {n_tiles}"
    # round-robin so per-core counts balance
    core_tiles = [[] for _ in range(NCORES)]
    for i, tl in enumerate(tiles):
        core_tiles[i % NCORES].append(tl)
    for i in range(NCORES):
        while len(core_tiles[i]) < S_MAX:
            core_tiles[i].append((0, np.full(TS, -1, np.int64)))

    slot2pix = np.full((NCORES, S), -1, np.int64)     # core, slot -> pixel
    tile_bucket = np.zeros((NCORES, S_MAX), np.int64)
    for i in range(NCORES):
        for t, (b, pads) in enumerate(core_tiles[i]):
            tile_bucket[i, t] = b
            slot2pix[i, t * TS : (t + 1) * TS] = pads
    pix2gslot = np.full(NPIX, -1, np.int64)           # pixel -> global AG2 row
    for i in range(NCORES):
        for s in range(S):
            p = slot2pix[i, s]
            if p >= 0:
                pix2gslot[p] = i * S_PAD + s
    assert (pix2gslot >= 0).all()

    # ---- index arrays ----
    pgidx = np.zeros((NCORES, PG_N), np.int64)        # patch gather
    for i in range(NCORES):
        for j in range(KK):
            dy, dx = j // 3 - 1, j % 3 - 1
            for s in range(S):
                p = slot2pix[i, s]
                if p < 0:
                    v = 0
                else:
                    y, xx = divmod(int(p), W)
                    v = _reflect(y + dy) * W + _reflect(xx + dx)
                pgidx[i, j * S + s] = v

    igidx = np.full((NCORES, IG_N), ZERO_SLOT, np.int64)  # image gather
    for i in range(NCORES):
        base = 8 * i * PW
        for j in range(STRIP_R * PW):
            r, cc = divmod(base + j, PW)
            y, xx = r - 1, cc - 1
            if 0 <= y < H and 0 <= xx < W:
                igidx[i, j] = pix2gslot[y * W + xx]

    xsidx = np.maximum(slot2pix, 0)                   # x0-at-slots gather

    # ---- weights ----
    def embw(emb):
        e = np.asarray(emb, np.float32).reshape(NTYPES, C, C * KK + 1)
        wf = e[:, :, : C * KK].reshape(NTYPES, C, C, KK)  # [t, o, c, kk]
        bias = e[:, :, -1]                                # [t, o]
        return wf, bias

    def conv1_bd(w1):
        # grouped (64,16,3,3) -> block-diag lhsT chunks [ci, kk, o]
        w1 = np.asarray(w1, np.float32)
        out = np.zeros((C, KK, C), np.float32)
        gs = C // GROUP
        for o in range(C):
            g = o // gs
            for cl in range(gs):
                out[g * gs + cl, :, o] = w1[o, cl].reshape(KK)
        return out

    repl = {}
    repl["ident"] = np.eye(128, dtype=np.float32).astype(BF16)
    x0pm = np.zeros((NPIX, 128), np.float32)
    x0pm[:, :C] = x.T
    repl["x0pm"] = x0pm.astype(BF16)
    for k, pre in ((1, "b1"), (2, "b2"), (3, "b3")):
        repl[f"w1bd{k}"] = conv1_bd(params[pre + "_w1"]).astype(BF16)
        repl[f"b1_{k}"] = np.asarray(params[pre + "_b1"], np.float32).reshape(C, 1)
        # w2 (64,64,1,1) -> lhsT [c, o]
        repl[f"w2t{k}"] = (
            np.asarray(params[pre + "_w2"], np.float32).reshape(C, C).T.copy()
        ).astype(BF16)
        repl[f"b2_{k}"] = np.asarray(params[pre + "_b2"], np.float32).reshape(C, 1)
    for k, cn in ((1, "c1"), (2, "c2"), (3, "c3")):
        cw = np.asarray(params[cn + "_w"], np.float32).reshape(C, C * (k + 1))
        # chunks [ci, j, o]
        repl[f"cw{k}"] = (
            cw.reshape(C, k + 1, C).transpose(2, 1, 0).copy()
        ).astype(BF16)
        repl[f"cb{k}"] = np.asarray(params[cn + "_b"], np.float32).reshape(C, 1)

    in_maps = []
    for i in range(NCORES):
        m = dict(repl)
        # x strip [64, 12, 66] zero-padded
        xs = np.zeros((C, STRIP_R, PW), np.float32)
        base = 8 * i * PW
        for j in range(STRIP_R * PW):
            r, cc = divmod(base + j, PW)
            y, xx = r - 1, cc - 1
            if 0 <= y < H and 0 <= xx < W:
                xs[:, j // PW, j % PW] = x[:, y * W + xx]
        m["xstrip"] = xs.astype(BF16)
        m["pgidx"] = _wrap_idx(pgidx[i], PG_N)
        m["igidx"] = _wrap_idx(igidx[i], IG_N)
        m["xsidx"] = _wrap_idx(xsidx[i], S)
        for k in (1, 2, 3):
            emb = params[f"b{k}_emb"]
            wf, bias = embw(emb)
            wloc = np.zeros((C, S_MAX, KK, C), np.float32)  # [ci, t, kk, o]
            bloc = np.zeros((C, S_MAX), np.float32)
            for t in range(S_MAX):
                b = tile_bucket[i, t]
                wloc[:, t, :, :] = wf[b].transpose(1, 2, 0)  # [c, kk, o]
                bloc[:, t] = bias[b]
            m[f"wloc{k}"] = wloc.astype(BF16)
            m[f"bloc{k}"] = bloc
        in_maps.append(m)

    return in_maps, slot2pix


def _emulate_core(m, k_blocks=3):
    """Pure-numpy mirror of the device program for ONE core, given its
    in_map plus the AllGather results (computed by _emulate below)."""
    raise NotImplementedError  # see _emulate


def _emulate(in_maps):
    """Numpy emulation of the full 8-core device program (validates all
    index/layout logic; mirrors device ops incl. bf16 rounding points)."""
    f32 = np.float32

    def bf(a):
        return a.astype(BF16)

    x0s = []
    for i in range(NCORES):
        g = _gather_np(in_maps[i]["x0pm"], in_maps[i]["xsidx"], S)
        x0s.append(g[:C].astype(f32))

    strip = [np.asarray(in_maps[i]["xstrip"], f32) for i in range(NCORES)]
    bslots = [[] for _ in range(NCORES)]
    oslot_prev = [None] * NCORES
    out = [None] * NCORES

    for k in (1, 2, 3):
        # conv1 on strips + relu
        ag1 = np.zeros((NPIX, 128), f32)
        for i in range(NCORES):
            w1 = np.asarray(in_maps[i][f"w1bd{k}"], f32)
            b1 = in_maps[i][f"b1_{k}"]
            ps = np.zeros((C, 8, 64), f32)
            for j in range(KK):
                dy, dx = j // 3, j % 3
                rhs = strip[i][:, dy : dy + 8, dx : dx + 64]
                ps += np.einsum("co,crw->orw", w1[:, j, :], rhs)
            c1 = np.maximum(ps.reshape(C, PIX_PER_CORE) + b1, 0)
            ag1[i * PIX_PER_CORE : (i + 1) * PIX_PER_CORE, :C] = bf(c1).T
        ag1 = bf(ag1)

        ag2 = np.zeros((NCORES * S_PAD, 128), f32)
        for i in range(NCORES):
            m = in_maps[i]
            patches = _gather_np(ag1, m["pgidx"], PG_N).astype(f32)  # [128, PG_N]
            wloc = np.asarray(m[f"wloc{k}"], f32)
            bloc = m[f"bloc{k}"]
            lrelu = np.zeros((C, S), f32)
            for t in range(S_MAX):
                acc = np.zeros((C, TS), f32)
                for j in range(KK):
                    rhs = patches[:C, j * S + t * TS : j * S + (t + 1) * TS]
                    acc += wloc[:, t, j, :].T @ rhs
                lrelu[:, t * TS : (t + 1) * TS] = np.maximum(
                    acc + bloc[:, t : t + 1], 0
                )
            lrelu = bf(lrelu).astype(f32)
            xs = x0s[i] if k == 1 else oslot_prev[i]
            w2t = np.asarray(m[f"w2t{k}"], f32)
            ps2 = w2t.T @ lrelu + xs  # identity-matmul residual
            bslot = np.maximum(ps2 + m[f"b2_{k}"], 0)
            bslot = bf(bslot).astype(f32)
            bslots[i].append(bslot)
            chain = [x0s[i]] + bslots[i]
            cwk = np.asarray(m[f"cw{k}"], f32)
            ps3 = np.zeros((C, S), f32)
            for j, rt in enumerate(chain):
                ps3 += cwk[:, j, :].T @ bf(rt.astype(f32)).astype(f32)
            ok = np.maximum(ps3 + m[f"cb{k}"], 0)
            if k == 3:
                out[i] = ok.astype(f32)
            else:
                okb = bf(ok)
                ag2[i * S_PAD : i * S_PAD + S, :C] = okb.T
                oslot_prev[i] = okb.astype(f32)
        if k < 3:
            ag2 = bf(ag2)
            for i in range(NCORES):
                g = _gather_np(ag2, in_maps[i]["igidx"], IG_N)
                strip[i] = (
                    g[:C, : STRIP_R * PW].reshape(C, STRIP_R, PW).astype(f32)
                )
    return out


def _gather_np(src, widx, n):
    """numpy mirror of dma_gather(transpose=True, elem=128): out[128, n]."""
    idx = widx[:16].T.reshape(-1)[:n].astype(np.int64)
    return np.asarray(src, np.float32)[idx].T.copy()  # [128, n]


def _assemble(outs, slot2pix):
    img = np.zeros((C, NPIX), np.float32)
    for i in range(NCORES):
        o = np.asarray(outs[i], np.float32)
        sel = slot2pix[i] >= 0
        img[:, slot2pix[i][sel]] = o[:, np.nonzero(sel)[0]]
    return img.reshape(1, C, H, W)


# ---------------------------------------------------------------------------
# bass graph
# ---------------------------------------------------------------------------


def _build_nc(trunc=0):
    import os
    import concourse.bass as bass
    import concourse.bacc as bacc
    import concourse.mybir as mybir
    import concourse.tile as tile

    dt = mybir.dt
    AF = mybir.ActivationFunctionType
    RG = [list(range(NCORES))]

    nc = bacc.Bacc(
        "TRN2",
        target_bir_lowering=False,
        debug=False,
        num_devices=NCORES,
    )

    # ---- parameters ----
    P = {}

    def param(name, shape, dtype):
        P[name] = nc.declare_dram_parameter(name, list(shape), dtype, False)

    param("xstrip", (C, STRIP_R, PW), dt.bfloat16)
    param("x0pm", (NPIX, 128), dt.bfloat16)
    param("ident", (128, 128), dt.bfloat16)
    param("pgidx", (128, PG_N // 16), dt.int16)
    param("igidx", (128, IG_N // 16), dt.int16)
    param("xsidx", (128, S // 16), dt.int16)
    for k in (1, 2, 3):
        param(f"w1bd{k}", (C, KK, C), dt.bfloat16)
        param(f"b1_{k}", (C, 1), dt.float32)
        param(f"w2t{k}", (C, C), dt.bfloat16)
        param(f"b2_{k}", (C, 1), dt.float32)
        param(f"cw{k}", (C, k + 1, C), dt.bfloat16)
        param(f"cb{k}", (C, 1), dt.float32)
        param(f"wloc{k}", (C, S_MAX, KK, C), dt.bfloat16)
        param(f"bloc{k}", (C, S_MAX), dt.float32)
    out_p = nc.declare_dram_parameter("out", [C, S], dt.float32, True)

    with tile.TileContext(nc) as tc:
        with (
            tc.tile_pool(name="wpool", bufs=1) as wpool,
            tc.tile_pool(name="work", bufs=1) as work,
            tc.tile_pool(name="psA", bufs=2, space=bass.MemorySpace.PSUM) as psA,
            tc.tile_pool(name="psB", bufs=1, space=bass.MemorySpace.PSUM) as psB,
            tc.tile_pool(name="dram", bufs=1, space=bass.MemorySpace.DRAM) as dram,
        ):
            # ---- load replicated/static data into SBUF ----
            sb = {}
            for name in P:
                if name == "x0pm":
                    continue  # DRAM gather source, not SBUF-resident
                t = wpool.tile(list(P[name].shape), P[name].dtype, name=f"sb_{name}")
                nc.sync.dma_start(t[:], P[name].ap())
                sb[name] = t

            ident = sb["ident"]

            # zero tile for AG2 zero rows
            zt = work.tile([64, 128], dt.bfloat16, name="zt")
            nc.vector.memset(zt[:], 0.0)

            # x0 at slots
            x0g = work.tile([128, 1, S], dt.bfloat16, name="x0g")
            nc.gpsimd.dma_gather(
                x0g[:], P["x0pm"].ap(), sb["xsidx"][:], S, S, 128, transpose=True
            )
            x0s = x0g[0:64, 0, :]

            strip_in = sb["xstrip"][:, :, :]
            bslot = {}
            chain = [x0s]

            for k in (1, 2, 3):
                # ---------- conv1 (strip) ----------
                ps1 = psB.tile([C, 8, 64], dt.float32, name=f"ps1_{k}", tag="ps1")
                for j in range(KK):
                    dy, dx = j // 3, j % 3
                    nc.tensor.matmul(
                        ps1[:],
                        sb[f"w1bd{k}"][:, j, :],
                        strip_in[:, dy : dy + 8, dx : dx + 64],
                        start=(j == 0),
                        stop=(j == KK - 1),
                    )
                c1 = work.tile([C, PIX_PER_CORE], dt.bfloat16, name=f"c1_{k}")
                nc.scalar.activation(
                    c1[:],
                    ps1.rearrange("p a b -> p (a b)"),
                    AF.Relu,
                    bias=sb[f"b1_{k}"][:],
                )

                # ---------- transpose to pixel-major + AG1 ----------
                pm1 = work.tile([128, 4, 128], dt.bfloat16, name=f"pm1_{k}")
                nc.vector.memset(pm1[:], 0.0)
                for cc in range(4):
                    pst = psA.tile([128, 64], dt.bfloat16, name=f"pst1_{k}_{cc}", tag="pst")
                    nc.tensor.transpose(
                        pst[:], c1[:, 128 * cc : 128 * (cc + 1)], ident[0:64, 0:64]
                    )
                    nc.scalar.activation(pm1[:, cc, 0:64], pst[:], AF.Copy)
                ag1in = dram.tile([PIX_PER_CORE, 128], dt.bfloat16, name=f"ag1in_{k}")
                nc.sync.dma_start(
                    ag1in.rearrange("(c r) e -> r c e", r=128), pm1[:]
                )
                c1pm = dram.tile(
                    [NPIX, 128], dt.bfloat16, name=f"c1pm_{k}", addr_space="Shared"
                )
                nc.gpsimd.collective_compute(
                    "AllGather",
                    mybir.AluOpType.bypass,
                    replica_groups=RG,
                    ins=[ag1in[:].opt()],
                    outs=[c1pm[:].opt()],
                )

                # ---------- patch gather ----------
                patches = work.tile([128, 1, PG_N], dt.bfloat16, name=f"patches_{k}")
                PGC = PG_N // 9  # noqa trunc-anchor
                for g3 in range(9):
                    nc.gpsimd.dma_gather(
                        patches[:, :, PGC * g3 : PGC * (g3 + 1)],
                        c1pm[:],
                        sb["pgidx"][:, (PGC // 16) * g3 : (PGC // 16) * (g3 + 1)],
                        PGC,
                        PGC,
                        128,
                        transpose=True,
                    )
                if trunc == 1:
                    tr = work.tile([C, S], dt.float32, name="trout")
                    nc.scalar.activation(tr[:], patches[0:64, 0, 0:S], AF.Copy)
                    nc.sync.dma_start(out_p.ap(), tr[:])
                    break

                # ---------- local conv per tile ----------
                lrelu = work.tile([C, S], dt.bfloat16, name=f"lrelu_{k}")
                for t in range(S_MAX):
                    psl = psA.tile([C, TS], dt.float32, name=f"psl_{k}_{t}", tag="psl")
                    for j in range(KK):
                        nc.tensor.matmul(
                            psl[:],
                            sb[f"wloc{k}"][:, t, j, :],
                            patches[0:64, 0, j * S + t * TS : j * S + (t + 1) * TS],
                            start=(j == 0),
                            stop=(j == KK - 1),
                        )
                    nc.scalar.activation(
                        lrelu[:, t * TS : (t + 1) * TS],
                        psl[:],
                        AF.Relu,
                        bias=sb[f"bloc{k}"][:, t : t + 1],
                    )

                if trunc == 2:
                    tr = work.tile([C, S], dt.float32, name="trout")
                    nc.scalar.activation(tr[:], lrelu[:], AF.Copy)
                    nc.sync.dma_start(out_p.ap(), tr[:])
                    break

                # ---------- conv2 + residual ----------
                xs = chain[-1] if k > 1 else x0s  # o_{k-1} slots (or x0)
                bs = work.tile([C, S], dt.bfloat16, name=f"bslot_{k}")
                for ch0 in range(0, S, 384):
                    sl = slice(ch0, ch0 + 384)
                    ps2 = psA.tile([C, 384], dt.float32, name=f"ps2_{k}_{ch0}", tag="ps2")
                    nc.tensor.matmul(
                        ps2[:], sb[f"w2t{k}"][:], lrelu[:, sl], start=True, stop=False
                    )
                    nc.tensor.matmul(
                        ps2[:], ident[0:64, 0:64], xs[:, sl], start=False, stop=True
                    )
                    nc.scalar.activation(
                        bs[:, sl], ps2[:], AF.Relu, bias=sb[f"b2_{k}"][:]
                    )
                bslot[k] = bs
                chain_k = [x0s] + [bslot[j][:, :] for j in range(1, k + 1)]

                # ---------- basic block ----------
                odt = dt.float32 if k == 3 else dt.bfloat16
                ok = work.tile([C, S], odt, name=f"oslot_{k}")
                for ch0 in range(0, S, 384):
                    sl = slice(ch0, ch0 + 384)
                    ps3 = psA.tile([C, 384], dt.float32, name=f"ps3_{k}_{ch0}", tag="ps2")
                    for j, rt in enumerate(chain_k):
                        nc.tensor.matmul(
                            ps3[:],
                            sb[f"cw{k}"][:, j, :],
                            rt[:, sl],
                            start=(j == 0),
                            stop=(j == len(chain_k) - 1),
                        )
                    nc.scalar.activation(
                        ok[:, sl], ps3[:], AF.Relu, bias=sb[f"cb{k}"][:]
                    )

                if k == 3 or trunc == 3:
                    if trunc == 3 and k != 3:
                        tr = work.tile([C, S], dt.float32, name="trout")
                        nc.scalar.activation(tr[:], ok[:], AF.Copy)
                        nc.sync.dma_start(out_p.ap(), tr[:])
                    else:
                        nc.sync.dma_start(out_p.ap(), ok[:])
                    break

                chain.append(ok[:, :])

                # ---------- AG2 + image gather for next block ----------
                pm2 = work.tile([128, 6, 128], dt.bfloat16, name=f"pm2_{k}")
                nc.vector.memset(pm2[:], 0.0)
                for cc in range(6):
                    pst2 = psA.tile([128, 64], dt.bfloat16, name=f"pst2_{k}_{cc}", tag="pst")
                    nc.tensor.transpose(
                        pst2[:], ok[:, 128 * cc : 128 * (cc + 1)], ident[0:64, 0:64]
                    )
                    nc.scalar.activation(pm2[:, cc, 0:64], pst2[:], AF.Copy)
                ag2in = dram.tile([S_PAD, 128], dt.bfloat16, name=f"ag2in_{k}")
                nc.sync.dma_start(
                    ag2in[0:S, :].rearrange("(c r) e -> r c e", r=128), pm2[:]
                )
                nc.sync.dma_start(ag2in[S:S_PAD, :], zt[:])
                ag2out = dram.tile(
                    [NCORES * S_PAD, 128],
                    dt.bfloat16,
                    name=f"ag2out_{k}",
                    addr_space="Shared",
                )
                nc.gpsimd.collective_compute(
                    "AllGather",
                    mybir.AluOpType.bypass,
                    replica_groups=RG,
                    ins=[ag2in[:].opt()],
                    outs=[ag2out[:].opt()],
                )
                ig = work.tile([128, 1, IG_N], dt.bfloat16, name=f"ig_{k}")
                nc.gpsimd.dma_gather(
                    ig[:], ag2out[:], sb["igidx"][:], IG_N, IG_N, 128, transpose=True
                )
                strip_in = ig[0:64, 0, 0 : STRIP_R * PW].rearrange(
                    "p (r c) -> p r c", c=PW
                )
                if trunc == 4:
                    tr = work.tile([C, S], dt.float32, name="trout")
                    nc.scalar.activation(tr[:], ig[0:64, 0, 0:S], AF.Copy)
                    nc.sync.dma_start(out_p.ap(), tr[:])
                    break

    nc.compile()
    return nc


_CACHE = {}


def kernel(**inputs):
    x = np.asarray(inputs["x"])
    params = {k: np.asarray(v) for k, v in inputs.items() if k not in ("x",)}
    buckets = params.pop("buckets")
    in_maps, slot2pix = _host_prep(x, buckets, params)

    import os
    trunc = int(os.environ.get("KTRUNC", "0"))
    if _CACHE.get("nc") is None:
        _CACHE["nc"] = _build_nc(trunc)
    nc = _CACHE["nc"]

    from concourse.bass_utils import run_bass_kernel_spmd

    trace = bool(int(os.environ.get("KTRACE", "0")))
    res = run_bass_kernel_spmd(
        nc, in_maps, core_ids=list(range(NCORES)), trace=trace
    )
    _CACHE["last_res"] = res
    outs = [res.results[i]["out"] for i in range(NCORES)]
    return _assemble(outs, slot2pix).astype(np.float32)


def kernel_emulate(**inputs):
    """Numpy emulation of the device program (for logic validation)."""
    x = np.asarray(inputs["x"])
    params = {k: np.asarray(v) for k, v in inputs.items() if k not in ("x",)}
    buckets = params.pop("buckets")
    in_maps, slot2pix = _host_prep(x, buckets, params)
    outs = _emulate(in_maps)
    return _assemble(outs, slot2pix).astype(np.float32)



# revision 8
# speedup vs baseline: 1.4188x; 1.4188x over previous
"""Trainium2 Bass kernel for nn_BlockBucket (3x eres_block + basic_block).

v2 strategy (v1 in kernel_v1_backup.py):
- Per-pixel dynamic conv via bucket-sorted slot tiles (as v1), but the patch
  data movement is restructured around SWDGE descriptor economics: descriptor
  GENERATION (~9ns/desc on the Q7) dominated v1, so all gathers are
  prepare_only-pregenerated (hidden under collectives) and triggered when
  their source lands; descriptor COUNT is cut 3x by a dual-row c1pm layout:
  row (y,x) = [c1(y,x) | c1(y,reflect(x-1))], 65 rows per image row (the 65th
  = column 62 copy handles reflect at x=63).  One 512B descriptor (elem=256,
  elem_step=128) then covers dx in {-1,0,+1} for one (slot,dy): 3 gathers of
  768 descs per block instead of 9.
- Local conv: per tile per dy one K=128 pair matmul + one K=64 single.
- PSUM-bank-batched activations; per-tile biases seeded via a K=16 matmul
  against a constant tile-indicator.
- Dummy AllGather first to absorb rank-launch skew; x0-at-slots host-prepped.
"""

import os
import sys

sys.path.insert(0, "/opt/trn_rl_repo")

import numpy as np
import ml_dtypes

BF16 = ml_dtypes.bfloat16

# problem constants
C = 64
H = W = 64
NPIX = H * W            # 4096
NTYPES = 72
KK = 9
GROUP = 4
NCORES = 8
TS = 64

# layout constants
CPMR = W + 1            # 65 c1pm rows per image row (64 cols + reflect-pad)
NCPM = H * CPMR         # 4160 c1pm rows
ROWS_PER_CORE = H // NCORES          # 8
CONTRIB = ROWS_PER_CORE * CPMR       # 520 c1pm rows per core
STRIP_R = 10            # strip rows: y-1 .. y+8
PW = 66                 # strip row width (x-1 .. x+64)
IG_N = 768              # ig gather num_idxs (>= STRIP_R*PW=660, %128==0)
S_AG = 768              # ag2 rows per core (>= S, %128, incl zero tail)
ZERO_SLOT = 704         # must be >= S; rows S..S_AG-1 are zeroed


def _reflect(v, n=64):
    if v < 0:
        return -v
    if v >= n:
        return 2 * n - 2 - v
    return v


def _wrap_idx(idx, n):
    """int16 index array -> [128, n//16] layout (j -> [j%16, j//16])."""
    assert len(idx) == n and n % 16 == 0
    blk = np.asarray(idx, np.int16).reshape(n // 16, 16).T
    return np.tile(blk, (8, 1))


def _host_prep(x, buckets, params):
    """Build per-core input maps + assembly info. All numpy."""
    x = np.asarray(x, np.float32).reshape(C, NPIX)
    bk = np.asarray(buckets, np.int64).reshape(NPIX)

    # ---- slot assignment ----
    tiles = []
    for t in range(NTYPES):
        pix = np.nonzero(bk == t)[0]
        for off in range(0, len(pix), TS):
            chunk = pix[off : off + TS]
            pad = np.full(TS, -1, np.int64)
            pad[: len(chunk)] = chunk
            tiles.append((t, pad))
    n_tiles = len(tiles)
    s_max = -(-n_tiles // NCORES)
    assert 7 <= s_max <= 12, s_max
    S = s_max * TS
    assert S <= ZERO_SLOT
    core_tiles = [[] for _ in range(NCORES)]
    for i, tl in enumerate(tiles):
        core_tiles[i % NCORES].append(tl)
    for i in range(NCORES):
        while len(core_tiles[i]) < s_max:
            core_tiles[i].append((0, np.full(TS, -1, np.int64)))

    slot2pix = np.full((NCORES, S), -1, np.int64)
    tile_bucket = np.zeros((NCORES, s_max), np.int64)
    for i in range(NCORES):
        for t, (b, pads) in enumerate(core_tiles[i]):
            tile_bucket[i, t] = b
            slot2pix[i, t * TS : (t + 1) * TS] = pads
    pix2gslot = np.full(NPIX, -1, np.int64)   # pixel -> global ag2 row
    for i in range(NCORES):
        for s in range(S):
            p = slot2pix[i, s]
            if p >= 0:
                pix2gslot[p] = i * S_AG + s
    assert (pix2gslot >= 0).all()

    # ---- index arrays ----
    # patch gathers: per dy one 768-idx gather; base row = reflect(y+dy)*65+x
    pgidx = np.zeros((NCORES, 3, IG_N), np.int64)
    for i in range(NCORES):
        for dyi, dy in enumerate((-1, 0, 1)):
            for s in range(S):
                p = slot2pix[i, s]
                if p < 0:
                    v = 0
                else:
                    y, xx = divmod(int(p), W)
                    v = _reflect(y + dy) * CPMR + xx
                pgidx[i, dyi, s] = v

    igidx = np.full((NCORES, IG_N), ZERO_SLOT, np.int64)
    for i in range(NCORES):
        for j in range(STRIP_R * PW):
            r, cc = divmod(j, PW)
            y, xx = 8 * i + r - 1, cc - 1
            if 0 <= y < H and 0 <= xx < W:
                igidx[i, j] = pix2gslot[y * W + xx]

    # ---- weights ----
    def embw(emb):
        e = np.asarray(emb, np.float32).reshape(NTYPES, C, C * KK + 1)
        wf = e[:, :, : C * KK].reshape(NTYPES, C, C, KK)  # [t, o, c, kk]
        bias = e[:, :, -1]                                # [t, o]
        return wf, bias

    def conv1_bd(w1):
        w1 = np.asarray(w1, np.float32)
        out = np.zeros((C, KK, C), np.float32)
        gs = C // GROUP
        for o in range(C):
            g = o // gs
            for cl in range(gs):
                out[g * gs + cl, :, o] = w1[o, cl].reshape(KK)
        return out

    SA = min(384, S)
    SB = S - SA
    selA = np.zeros((16, SA), np.float32)
    selB = np.zeros((16, max(SB, 1)), np.float32)
    for s in range(SA):
        selA[s // TS, s] = 1.0
    for s in range(SB):
        selB[(s + SA) // TS, s] = 1.0

    repl = {}
    repl["ident"] = np.eye(128, dtype=np.float32).astype(BF16)
    repl["selA"] = selA.astype(BF16)
    repl["selB"] = selB.astype(BF16)
    for k, pre in ((1, "b1"), (2, "b2"), (3, "b3")):
        repl[f"w1bd{k}"] = conv1_bd(params[pre + "_w1"]).astype(BF16)
        repl[f"b1_{k}"] = np.asarray(params[pre + "_b1"], np.float32).reshape(C, 1)
        repl[f"w2t{k}"] = (
            np.asarray(params[pre + "_w2"], np.float32).reshape(C, C).T.copy()
        ).astype(BF16)
        repl[f"b2_{k}"] = np.asarray(params[pre + "_b2"], np.float32).reshape(C, 1)
    for k, cn in ((1, "c1"), (2, "c2"), (3, "c3")):
        cw = np.asarray(params[cn + "_w"], np.float32).reshape(C, C * (k + 1))
        repl[f"cw{k}"] = (
            cw.reshape(C, k + 1, C).transpose(2, 1, 0).copy()
        ).astype(BF16)
        repl[f"cb{k}"] = np.asarray(params[cn + "_b"], np.float32).reshape(C, 1)

    in_maps = []
    for i in range(NCORES):
        m = dict(repl)
        # strip for block 1: rows y-1..y+8, zero-padded outside image
        xs = np.zeros((C, STRIP_R, PW), np.float32)
        for j in range(STRIP_R * PW):
            r, cc = divmod(j, PW)
            y, xx = 8 * i + r - 1, cc - 1
            if 0 <= y < H and 0 <= xx < W:
                xs[:, r, cc] = x[:, y * W + xx]
        m["xstrip"] = xs.astype(BF16)
        x0s = np.zeros((C, S), np.float32)
        sel = slot2pix[i] >= 0
        x0s[:, sel] = x[:, slot2pix[i][sel]]
        m["x0slots"] = x0s.astype(BF16)
        for dyi in range(3):
            m[f"pg{dyi}"] = _wrap_idx(pgidx[i, dyi], IG_N)
        m["igidx"] = _wrap_idx(igidx[i], IG_N)
        for k in (1, 2, 3):
            wf, bias = embw(params[f"b{k}_emb"])
            wp = np.zeros((128, s_max, 3, C), np.float32)
            ws = np.zeros((C, s_max, 3, C), np.float32)
            bt = np.zeros((16, C), np.float32)
            for t in range(s_max):
                b = tile_bucket[i, t]
                bt[t] = bias[b]
                for dyi, dy in enumerate((-1, 0, 1)):
                    kk0 = (dy + 1) * 3 + 1   # dx = 0
                    kkm = (dy + 1) * 3 + 0   # dx = -1
                    kkp = (dy + 1) * 3 + 2   # dx = +1
                    wp[0:C, t, dyi, :] = wf[b, :, :, kk0].T
                    wp[C:128, t, dyi, :] = wf[b, :, :, kkm].T
                    ws[:, t, dyi, :] = wf[b, :, :, kkp].T
            m[f"wlocP{k}"] = wp.astype(BF16)
            m[f"wlocS{k}"] = ws.astype(BF16)
            m[f"blocT{k}"] = bt.astype(BF16)
        in_maps.append(m)

    meta = dict(s_max=s_max, S=S, SA=SA, SB=SB)
    return in_maps, slot2pix, meta


# ---------------------------------------------------------------------------
# numpy emulation (mirrors the device program, incl bf16 rounding points)
# ---------------------------------------------------------------------------


def _gather_np_t(src, widx, n, elem, step):
    """mirror of dma_gather(transpose=True): out[128*ng, n] where the desc at
    idx v covers src flat bytes [v*step : v*step+elem] (elems, bf16)."""
    idx = widx[:16].T.reshape(-1)[:n].astype(np.int64)
    flat = np.asarray(src, np.float32).reshape(-1)
    ng = elem // 128
    out = np.zeros((128, ng, n), np.float32)
    for j, v in enumerate(idx):
        seg = flat[v * step : v * step + elem]
        out[:, :, j] = seg.reshape(ng, 128).T
    return out


def _emulate(in_maps, meta):
    f32 = np.float32
    S, SA, SB, s_max = meta["S"], meta["SA"], meta["SB"], meta["s_max"]

    def bf(a):
        return np.asarray(a, BF16).astype(f32)

    strip = [np.asarray(in_maps[i]["xstrip"], f32) for i in range(NCORES)]
    x0s = [np.asarray(in_maps[i]["x0slots"], f32) for i in range(NCORES)]
    bslots = [[] for _ in range(NCORES)]
    ok_prev = [None] * NCORES
    out = [None] * NCORES

    for k in (1, 2, 3):
        # conv1 on strips + relu -> dual-row c1pm
        c1pm = np.zeros((NCPM, 128), f32)
        c1s = []
        for i in range(NCORES):
            w1 = np.asarray(in_maps[i][f"w1bd{k}"], f32)
            b1 = in_maps[i][f"b1_{k}"]
            ps = np.zeros((C, 8, 64), f32)
            for j in range(KK):
                dy, dx = j // 3, j % 3
                rhs = strip[i][:, dy : dy + 8, dx : dx + 64]
                ps += np.einsum("co,crw->orw", w1[:, j, :], rhs)
            c1 = np.maximum(ps + b1.reshape(C, 1, 1), 0)
            c1 = bf(c1)
            c1s.append(c1)
            for r in range(8):
                gy = 8 * i + r
                rows = gy * CPMR
                c1pm[rows : rows + 64, 0:64] = c1[:, r, :].T
                c1pm[rows + 1 : rows + 64, 64:128] = c1[:, r, 0:63].T
                c1pm[rows, 64:128] = c1[:, r, 1]
                c1pm[rows + 64, 0:64] = c1[:, r, 62]
                c1pm[rows + 64, 64:128] = c1[:, r, 61]
        c1pm = bf(c1pm)

        ag2 = np.zeros((NCORES * S_AG, 128), f32)
        for i in range(NCORES):
            m = in_maps[i]
            psl = np.zeros((C, S), f32)
            bt = np.asarray(m[f"blocT{k}"], f32)
            selA = np.asarray(m["selA"], f32)
            selB = np.asarray(m["selB"], f32)
            psl[:, 0:SA] = bt.T @ selA
            if SB:
                psl[:, SA:S] = bt.T @ selB
            wp = np.asarray(m[f"wlocP{k}"], f32)
            ws = np.asarray(m[f"wlocS{k}"], f32)
            for dyi in range(3):
                pat = _gather_np_t(c1pm, m[f"pg{dyi}"], IG_N, 256, 128)
                for t in range(s_max):
                    sl = slice(t * TS, (t + 1) * TS)
                    psl[:, sl] += wp[:, t, dyi, :].T @ pat[:, 0, sl]
                    psl[:, sl] += ws[:, t, dyi, :].T @ pat[0:64, 1, sl]
            lrelu = bf(np.maximum(psl, 0))
            xs = x0s[i] if k == 1 else ok_prev[i]
            w2t = np.asarray(m[f"w2t{k}"], f32)
            bs = np.maximum(w2t.T @ lrelu + xs + m[f"b2_{k}"], 0)
            bs = bf(bs)
            bslots[i].append(bs)
            chain = [x0s[i]] + bslots[i]
            cwk = np.asarray(m[f"cw{k}"], f32)
            ps3 = np.zeros((C, S), f32)
            for j, rt in enumerate(chain):
                ps3 += cwk[:, j, :].T @ rt
            ok = np.maximum(ps3 + m[f"cb{k}"], 0)
            if k == 3:
                out[i] = ok.astype(f32)
            else:
                okb = bf(ok)
                ag2[i * S_AG : i * S_AG + S, 0:64] = okb.T
                ok_prev[i] = okb
        if k < 3:
            ag2 = bf(ag2)
            for i in range(NCORES):
                g = _gather_np_t(ag2, in_maps[i]["igidx"], IG_N, 128, 128)
                strip[i] = (
                    g[0:64, 0, : STRIP_R * PW].reshape(C, STRIP_R, PW)
                )
    return out


def _assemble(outs, slot2pix):
    img = np.zeros((C, NPIX), np.float32)
    for i in range(NCORES):
        o = np.asarray(outs[i], np.float32)
        sel = slot2pix[i] >= 0
        img[:, slot2pix[i][sel]] = o[:, np.nonzero(sel)[0]]
    return img.reshape(1, C, H, W)


# ---------------------------------------------------------------------------
# bass graph
# ---------------------------------------------------------------------------


def _build_nc(meta, trunc=0):
    import concourse.bass as bass
    import concourse.bacc as bacc
    import concourse.mybir as mybir
    import concourse.tile as tile
    from concourse.ap import AP

    dt = mybir.dt
    AF = mybir.ActivationFunctionType
    RG = [list(range(NCORES))]
    S, SA, SB, s_max = meta["S"], meta["SA"], meta["SB"], meta["s_max"]

    nc = bacc.Bacc(
        "TRN2",
        target_bir_lowering=False,
        debug=False,
        num_devices=NCORES,
    )

    P = {}

    def param(name, shape, dtype):
        P[name] = nc.declare_dram_parameter(name, list(shape), dtype, False)

    # order matters only for readability; SBUF loads are emitted in priority
    # order below.
    param("ident", (128, 128), dt.bfloat16)
    param("xstrip", (C, STRIP_R, PW), dt.bfloat16)
    param("x0slots", (C, S), dt.bfloat16)
    param("selA", (16, SA), dt.bfloat16)
    param("selB", (16, max(SB, 1)), dt.bfloat16)
    for dyi in range(3):
        param(f"pg{dyi}", (128, IG_N // 16), dt.int16)
    param("igidx", (128, IG_N // 16), dt.int16)
    for k in (1, 2, 3):
        param(f"w1bd{k}", (C, KK, C), dt.bfloat16)
        param(f"b1_{k}", (C, 1), dt.float32)
        param(f"wlocP{k}", (128, s_max, 3, C), dt.bfloat16)
        param(f"wlocS{k}", (C, s_max, 3, C), dt.bfloat16)
        param(f"blocT{k}", (16, C), dt.bfloat16)
        param(f"w2t{k}", (C, C), dt.bfloat16)
        param(f"b2_{k}", (C, 1), dt.float32)
        param(f"cw{k}", (C, k + 1, C), dt.bfloat16)
        param(f"cb{k}", (C, 1), dt.float32)
    out_p = nc.declare_dram_parameter("out", [C, S], dt.float32, True)

    with tile.TileContext(nc) as tc:
        with (
            tc.tile_pool(name="wpool", bufs=1) as wpool,
            tc.tile_pool(name="work", bufs=1) as work,
            tc.tile_pool(name="rot", bufs=2) as rot,
            tc.tile_pool(name="psA", bufs=1, space=bass.MemorySpace.PSUM) as psA,
            tc.tile_pool(name="psT", bufs=2, space=bass.MemorySpace.PSUM) as psT,
            tc.tile_pool(name="dram", bufs=1, space=bass.MemorySpace.DRAM) as dram,
        ):
            # ---- dummy collective: absorb rank-launch skew ----
            dum_in = dram.tile([16, 64], dt.bfloat16, name="dum_in")
            dum_out = dram.tile(
                [128, 64], dt.bfloat16, name="dum_out", addr_space="Shared"
            )
            nc.gpsimd.collective_compute(
                "AllGather",
                mybir.AluOpType.bypass,
                replica_groups=RG,
                ins=[dum_in[:].opt()],
                outs=[dum_out[:].opt()],
            )

            # ---- param loads (block-1-critical first) ----
            sb = {}
            load_order = [
                "ident", "xstrip", "w1bd1", "b1_1",
                "pg0", "pg1", "pg2", "igidx",
                "wlocP1", "wlocS1", "blocT1", "selA", "selB",
                "x0slots", "w2t1", "b2_1", "cw1", "cb1",
            ]
            for k in (2, 3):
                load_order += [f"w1bd{k}", f"b1_{k}", f"wlocP{k}", f"wlocS{k}",
                               f"blocT{k}", f"w2t{k}", f"b2_{k}", f"cw{k}",
                               f"cb{k}"]
            for name in load_order:
                t = wpool.tile(list(P[name].shape), P[name].dtype, name=f"sb_{name}")
                nc.sync.dma_start(t[:], P[name].ap())
                sb[name] = t

            ident = sb["ident"]
            zt = work.tile([64, 128], dt.bfloat16, name="zt")
            nc.vector.memset(zt[:], 0.0)

            x0s = sb["x0slots"][:, :]
            strip_in = sb["xstrip"][:, :, :]
            bslot = {}
            chain = [x0s]

            # dram tiles per block
            ag1in = {}
            c1pm = {}
            ag2in = {}
            ag2out = {}
            for k in (1, 2, 3):
                ag1in[k] = dram.tile([CONTRIB, 128], dt.bfloat16,
                                     name=f"ag1in_{k}")
                c1pm[k] = dram.tile([NCPM, 128], dt.bfloat16,
                                    name=f"c1pm_{k}", addr_space="Shared")
                if k < 3:
                    ag2in[k] = dram.tile([S_AG, 128], dt.bfloat16,
                                         name=f"ag2in_{k}")
                    ag2out[k] = dram.tile([NCORES * S_AG, 128], dt.bfloat16,
                                          name=f"ag2out_{k}",
                                          addr_space="Shared")

            # prep bookkeeping: emit preps in trigger order
            sems = {}
            use_prep = bool(int(os.environ.get("KPREP", "1")))

            def prep_patches(k):
                pats = []
                for dyi in range(3):
                    pt = rot.tile([128, 2, IG_N], dt.bfloat16,
                                  name=f"pat{k}_{dyi}", tag=f"pat{dyi}")
                    ov = AP(
                        tensor=c1pm[k][:].tensor, offset=0,
                        ap=[[128, NCPM - 1], [1, 256]],
                    )
                    if use_prep:
                        sem = nc.alloc_semaphore(f"pg{k}{dyi}")
                        sems[(k, dyi)] = sem
                        nc.gpsimd.dma_gather(
                            pt[:], ov, sb[f"pg{dyi}"][:], IG_N, IG_N, 256,
                            elem_step=128, transpose=True,
                            prepare_only=True, sem=sem,
                        )
                    else:
                        pats.append((pt, ov))
                        continue
                    pats.append(pt)
                return pats

            def fire_patches(k, pats):
                if use_prep:
                    nc.gpsimd.trigger_dma(count=3)
                    return pats
                real = []
                for dyi, (pt, ov) in enumerate(pats):
                    nc.gpsimd.dma_gather(
                        pt[:], ov, sb[f"pg{dyi}"][:], IG_N, IG_N, 256,
                        elem_step=128, transpose=True,
                    )
                    real.append(pt)
                return real

            def prep_ig(k):
                it = rot.tile([128, 1, IG_N], dt.bfloat16,
                              name=f"ig_{k}", tag="igt")
                if use_prep:
                    sem = nc.alloc_semaphore(f"ig{k}")
                    sems[("ig", k)] = sem
                    nc.gpsimd.dma_gather(
                        it[:], ag2out[k - 1][:], sb["igidx"][:], IG_N, IG_N,
                        128, transpose=True, prepare_only=True, sem=sem,
                    )
                return it

            def fire_ig(k, it):
                if use_prep:
                    nc.gpsimd.trigger_dma(count=1)
                else:
                    nc.gpsimd.dma_gather(
                        it[:], ag2out[k - 1][:], sb["igidx"][:], IG_N, IG_N,
                        128, transpose=True,
                    )

            # preps for block 1 + ig2 (generated during startup/dummy AG)
            pats = prep_patches(1)
            ig_next = prep_ig(2)

            for k in (1, 2, 3):
                # ---------- conv1 ----------
                ps1 = psA.tile([C, 8, 64], dt.float32, name=f"ps1_{k}",
                               tag="ps1")
                for j in range(KK):
                    dy, dx = j // 3, j % 3
                    nc.tensor.matmul(
                        ps1[:],
                        sb[f"w1bd{k}"][:, j, :],
                        strip_in[:, dy : dy + 8, dx : dx + 64],
                        start=(j == 0),
                        stop=(j == KK - 1),
                    )
                c1 = rot.tile([C, 8, 64], dt.bfloat16, name=f"c1_{k}",
                              tag="c1")
                nc.scalar.activation(
                    c1.rearrange("p a b -> p (a b)"),
                    ps1.rearrange("p a b -> p (a b)"),
                    AF.Relu,
                    bias=sb[f"b1_{k}"][:],
                )

                # left-shifted copy (reflect at x=0)
                lc1 = rot.tile([C, 8, 64], dt.bfloat16, name=f"lc1_{k}",
                               tag="lc1")
                nc.vector.tensor_copy(lc1[:, :, 1:64], c1[:, :, 0:63])
                nc.vector.tensor_copy(lc1[:, :, 0:1], c1[:, :, 1:2])

                # transposes -> pixel-major pm1 [128, 4, 128] = [self | left]
                c1f = c1.rearrange("p a b -> p (a b)")
                lc1f = lc1.rearrange("p a b -> p (a b)")
                pstS = psT.tile([128, 4, 64], dt.bfloat16, name=f"pstS_{k}",
                                tag="pst")
                pstL = psT.tile([128, 4, 64], dt.bfloat16, name=f"pstL_{k}",
                                tag="pst")
                for cc in range(4):
                    nc.tensor.transpose(
                        pstS[:, cc, :], c1f[:, 128 * cc : 128 * (cc + 1)],
                        ident[0:64, 0:64],
                    )
                    nc.tensor.transpose(
                        pstL[:, cc, :], lc1f[:, 128 * cc : 128 * (cc + 1)],
                        ident[0:64, 0:64],
                    )
                pm1 = rot.tile([128, 4, 128], dt.bfloat16, name=f"pm1_{k}",
                               tag="pm1")
                nc.scalar.activation(pm1[:, :, 0:64], pstS[:], AF.Copy)
                nc.vector.tensor_copy(pm1[:, :, 64:128], pstL[:])

                # ag1in writes: rows (2*r2+rlo)*65+c <- pm1[rlo*64+c, r2, :]
                ag1t = ag1in[k][:].tensor
                for rlo in range(2):
                    dst = AP(
                        tensor=ag1t, offset=rlo * CPMR * 128,
                        ap=[[128, 64], [2 * CPMR * 128, 4], [1, 128]],
                    )
                    nc.sync.dma_start(dst, pm1[rlo * 64 : rlo * 64 + 64, :, :])
                    # pad row (r,64) <- pm1 entry at column 62
                    dstp = AP(
                        tensor=ag1t, offset=(rlo * CPMR + 64) * 128,
                        ap=[[0, 1], [2 * CPMR * 128, 4], [1, 128]],
                    )
                    nc.sync.dma_start(
                        dstp, pm1[rlo * 64 + 62 : rlo * 64 + 63, :, :]
                    )

                # ---------- AG1 ----------
                nc.gpsimd.collective_compute(
                    "AllGather",
                    mybir.AluOpType.bypass,
                    replica_groups=RG,
                    ins=[ag1in[k][:].opt()],
                    outs=[c1pm[k][:].opt()],
                )
                # during AG1 flight: generate next ig descriptors (k<3)
                if k < 3 and k == 2:
                    ig_next = prep_ig(3)
                # fire the 3 patch gathers once c1pm lands
                pats = fire_patches(k, pats)

                # ---------- local conv ----------
                psl_a = psA.tile([C, SA], dt.float32, name=f"psla_{k}",
                                 tag="psla")
                psl_b = psA.tile([C, SB], dt.float32, name=f"pslb_{k}",
                                 tag="pslb")
                nc.tensor.matmul(psl_a[:], sb[f"blocT{k}"][:], sb["selA"][:],
                                 start=True, stop=False)
                nc.tensor.matmul(psl_b[:], sb[f"blocT{k}"][:],
                                 sb["selB"][:, 0:SB], start=True, stop=False)
                for dyi in range(3):
                    last = dyi == 2
                    for t in range(s_max):
                        sl = slice(t * TS, (t + 1) * TS)
                        if t * TS < SA:
                            dst = psl_a[:, sl]
                        else:
                            dst = psl_b[:, t * TS - SA : (t + 1) * TS - SA]
                        nc.tensor.matmul(
                            dst, sb[f"wlocP{k}"][:, t, dyi, :],
                            pats[dyi][:, 0, sl], start=False, stop=False,
                        )
                        nc.tensor.matmul(
                            dst, sb[f"wlocS{k}"][:, t, dyi, :],
                            pats[dyi][0:64, 1, sl], start=False, stop=last,
                        )
                lrelu = rot.tile([C, S], dt.bfloat16, name=f"lrelu_{k}",
                                 tag="lrelu")
                nc.scalar.activation(lrelu[:, 0:SA], psl_a[:], AF.Relu)
                nc.scalar.activation(lrelu[:, SA:S], psl_b[:], AF.Relu)

                if trunc == 2:
                    tr = work.tile([C, S], dt.float32, name="trout")
                    nc.scalar.activation(tr[:], lrelu[:], AF.Copy)
                    nc.sync.dma_start(out_p.ap(), tr[:])
                    break

                # ---------- conv2 + residual ----------
                xs = chain[-1] if k > 1 else x0s
                bs = work.tile([C, S], dt.bfloat16, name=f"bslot_{k}")
                for ch0, chw in ((0, SA), (SA, SB)):
                    sl = slice(ch0, ch0 + chw)
                    ps2 = psA.tile([C, chw], dt.float32, name=f"ps2_{k}_{ch0}",
                                   tag="ps2")
                    nc.tensor.matmul(ps2[:], sb[f"w2t{k}"][:], lrelu[:, sl],
                                     start=True, stop=False)
                    nc.tensor.matmul(ps2[:], ident[0:64, 0:64], xs[:, sl],
                                     start=False, stop=True)
                    nc.scalar.activation(bs[:, sl], ps2[:], AF.Relu,
                                         bias=sb[f"b2_{k}"][:])
                bslot[k] = bs
                chain_k = [x0s] + [bslot[j][:, :] for j in range(1, k + 1)]

                # ---------- basic block ----------
                odt = dt.float32 if k == 3 else dt.bfloat16
                ok = work.tile([C, S], odt, name=f"oslot_{k}")
                for ch0, chw in ((0, SA), (SA, SB)):
                    sl = slice(ch0, ch0 + chw)
                    ps3 = psA.tile([C, chw], dt.float32, name=f"ps3_{k}_{ch0}",
                                   tag="ps3")
                    for j, rt in enumerate(chain_k):
                        nc.tensor.matmul(
                            ps3[:], sb[f"cw{k}"][:, j, :], rt[:, sl],
                            start=(j == 0), stop=(j == len(chain_k) - 1),
                        )
                    nc.scalar.activation(ok[:, sl], ps3[:], AF.Relu,
                                         bias=sb[f"cb{k}"][:])

                if k == 3:
                    nc.sync.dma_start(out_p.ap(), ok[:])
                    break

                chain.append(ok[:, :])

                # ---------- ag2 build + AG2 ----------
                nfull = S // 128          # full 128-row groups
                rem = S - nfull * 128     # 0 or 64
                ngrp = nfull + (1 if rem else 0)
                pm2 = rot.tile([128, ngrp, 64], dt.bfloat16, name=f"pm2_{k}",
                               tag="pm2")
                pst2 = psT.tile([128, ngrp, 64], dt.bfloat16, name=f"pst2_{k}",
                                tag="pst")
                for cc in range(nfull):
                    nc.tensor.transpose(
                        pst2[:, cc, :], ok[:, 128 * cc : 128 * (cc + 1)],
                        ident[0:64, 0:64],
                    )
                if rem:
                    nc.tensor.transpose(
                        pst2[0:rem, nfull, :], ok[:, nfull * 128 : S],
                        ident[0:64, 0:64],
                    )
                nc.vector.tensor_copy(pm2[:], pst2[:])
                nc.sync.dma_start(
                    ag2in[k][0 : nfull * 128, 0:64].rearrange(
                        "(c r) e -> r c e", r=128
                    ),
                    pm2[:, 0:nfull, :],
                )
                if rem:
                    nc.sync.dma_start(
                        ag2in[k][nfull * 128 : S, 0:64], pm2[0:rem, nfull, :]
                    )
                nc.sync.dma_start(ag2in[k][ZERO_SLOT:S_AG, :], zt[:])
                nc.gpsimd.collective_compute(
                    "AllGather",
                    mybir.AluOpType.bypass,
                    replica_groups=RG,
                    ins=[ag2in[k][:].opt()],
                    outs=[ag2out[k][:].opt()],
                )
                # during AG2 flight: generate next block's patch descs
                pats = prep_patches(k + 1)
                # fire ig gather once ag2out lands
                fire_ig(k + 1, ig_next)
                strip_in = ig_next[0:64, 0, 0 : STRIP_R * PW].rearrange(
                    "p (r c) -> p r c", c=PW
                )

    nc.compile()
    return nc


_CACHE = {}


def kernel(**inputs):
    x = np.asarray(inputs["x"])
    params = {k: np.asarray(v) for k, v in inputs.items() if k not in ("x",)}
    buckets = params.pop("buckets")
    in_maps, slot2pix, meta = _host_prep(x, buckets, params)

    trunc = int(os.environ.get("KTRUNC", "0"))
    key = (meta["s_max"], trunc, os.environ.get("KPREP", "1"))
    if _CACHE.get(key) is None:
        _CACHE[key] = _build_nc(meta, trunc)
    nc = _CACHE[key]

    from concourse.bass_utils import run_bass_kernel_spmd

    trace = bool(int(os.environ.get("KTRACE", "0")))
    res = run_bass_kernel_spmd(
        nc, in_maps, core_ids=list(range(NCORES)), trace=trace
    )
    _CACHE["last_res"] = res
    outs = [res.results[i]["out"] for i in range(NCORES)]
    return _assemble(outs, slot2pix).astype(np.float32)


def kernel_emulate(**inputs):
    x = np.asarray(inputs["x"])
    params = {k: np.asarray(v) for k, v in inputs.items() if k not in ("x",)}
    buckets = params.pop("buckets")
    in_maps, slot2pix, meta = _host_prep(x, buckets, params)
    outs = _emulate(in_maps, meta)
    return _assemble(outs, slot2pix).astype(np.float32)
